# revision 6
# baseline (speedup 1.0000x reference)
"""Trainium2 Bass kernel for nn_CausalFFNN (pairwise relu-MLP scores), v2.

Computes: Hn = relu(relu(E@W1+b1)@W2+b2)
          logits[i,j] = relu(Hn[i]@Wa + Hn[j]@Wb + bp1) @ Wp2 + bp2
          out = softplus(logits), diag = 0
Sharding: i-rows split across 8 cores (128 rows each); weights + full E
replicated. Each core computes a (128, 1024) output slab.

v2 vs baseline: col-group-concurrent pairwise matmuls (4 streams), R-tile
production load-balanced across DVE/ACT/GPSIMD, single-pass Softplus drain
with DMA-compacted PSUM strips, f16 output.
"""
import sys
import os
import tempfile
import numpy as np

os.environ["NEURON_COMPILE_CACHE_URL"] = tempfile.mkdtemp(prefix="neuron-cache-")

for _p in ("/opt/trn_rl_repo", "/root/.axon_site/_ro/trn_rl_repo"):
    if os.path.isdir(_p) and _p not in sys.path:
        sys.path.insert(0, _p)

N, D, HID = 1024, 512, 256
NCORE = 8
SLAB = N // NCORE          # 128 i-rows per core
P = 128
G = 8                      # i's per PSUM col strip
HSUB = HID // 16           # 16 h-components per chunk per i
NT = HID // HSUB           # 16 chunks
NR = 4                     # rounds
NS = 4                     # col strips
N_MM = 512                 # moving free-dim per pairwise matmul (PSUM bank cap)
USE_GPSIMD = False

_CACHE = {}


class _Balancer:
    """Greedy per-engine busy-time balancer (build-time scheduling)."""

    def __init__(self, init):
        self.busy = dict(init)

    def pick(self, costs):
        e = min(costs, key=lambda e: self.busy[e] + costs[e])
        self.busy[e] += costs[e]
        return e


def _build_program(repeat=1):
    import concourse.bacc as bacc
    import concourse.mybir as mybir
    from concourse.tile import TileContext

    F32 = mybir.dt.float32
    F32R = mybir.dt.float32r
    F16 = mybir.dt.float16
    AF = mybir.ActivationFunctionType
    ALU = mybir.AluOpType

    nc = bacc.Bacc("TRN2", target_bir_lowering=False, debug=False)

    dET = nc.dram_tensor("ET", [D, N], F32R, kind="ExternalInput")
    dETs = nc.dram_tensor("ETs", [D, SLAB], F32R, kind="ExternalInput")
    dW1s = nc.dram_tensor("W1s", [P, 4 * HID], F32R, kind="ExternalInput")
    dW2s = nc.dram_tensor("W2s", [P, 2 * HID], F32R, kind="ExternalInput")
    dWas = nc.dram_tensor("Was", [P, 2 * HID], F32R, kind="ExternalInput")
    dWbrep = nc.dram_tensor("Wbrep", [NT, 2, P, P], F16, kind="ExternalInput")
    db1 = nc.dram_tensor("b1c", [P, 2], F32, kind="ExternalInput")
    db2 = nc.dram_tensor("b2c", [P, 2], F32, kind="ExternalInput")
    dbp1 = nc.dram_tensor("bp1c", [P, 2], F32, kind="ExternalInput")
    dbp2 = nc.dram_tensor("bp2c", [P, 1], F32, kind="ExternalInput")
    dWst = nc.dram_tensor("Wst", [P, NT * 32], F16, kind="ExternalInput")
    dY = nc.dram_tensor("Y", [SLAB, N], F16, kind="ExternalOutput")

    with TileContext(nc) as tc:
        with tc.tile_pool(name="const", bufs=1) as cpool, \
             tc.tile_pool(name="work", bufs=1) as wpool, \
             tc.tile_pool(name="rpool", bufs=16) as rpool, \
             tc.tile_pool(name="dpool", bufs=1, space="DRAM") as dpool:

            # ---------- load constants (small/urgent first) ----------
            W1s = cpool.tile([P, 4 * HID], F32R, tag="W1s")
            nc.sync.dma_start(W1s[:], dW1s.ap())
            W2s = cpool.tile([P, 2 * HID], F32R, tag="W2s")
            nc.sync.dma_start(W2s[:], dW2s.ap())
            Was = cpool.tile([P, 2 * HID], F32R, tag="Was")
            nc.sync.dma_start(Was[:], dWas.ap())
            b1c = cpool.tile([P, 2], F32, tag="b1c")
            nc.sync.dma_start(b1c[:], db1.ap())
            b2c = cpool.tile([P, 2], F32, tag="b2c")
            nc.sync.dma_start(b2c[:], db2.ap())
            bp1c = cpool.tile([P, 2], F32, tag="bp1c")
            nc.sync.dma_start(bp1c[:], dbp1.ap())
            bp2c = cpool.tile([P, 1], F32, tag="bp2c")
            nc.sync.dma_start(bp2c[:], dbp2.ap())
            Wst = cpool.tile([P, NT * 32], F16, tag="Wst")
            nc.sync.dma_start(Wst[:], dWst.ap())
            ETs = cpool.tile([P, 4 * SLAB], F32R, tag="ETs")
            for kd in range(4):
                nc.sync.dma_start(ETs[:, kd * SLAB:(kd + 1) * SLAB],
                                  dETs.ap()[kd * P:(kd + 1) * P, :])
            ET = cpool.tile([P, 4 * N], F32R, tag="ET")
            for kd in range(4):
                nc.sync.dma_start(ET[:, kd * N:(kd + 1) * N],
                                  dET.ap()[kd * P:(kd + 1) * P, :])
            Wbrep = cpool.tile([P, NT * 2 * P], F16, tag="Wbrep")
            nc.sync.dma_start(
                Wbrep[:].rearrange("p (t kh m) -> p t kh m", kh=2, m=P),
                dWbrep.ap().rearrange("t kh p m -> p t kh m"))

            ATd = dpool.tile([HID, SLAB], F32, tag="ATd")
            H1T = wpool.tile([P, 2 * N], F32R, tag="H1T")
            HnT = wpool.tile([P, 2 * N], F16, tag="HnT")
            H1Ts = wpool.tile([P, 2 * SLAB], F32R, tag="H1Ts")
            HnTs = wpool.tile([P, 2 * SLAB], F32R, tag="HnTs")
            ATs = wpool.tile([P, 2 * SLAB], F32, tag="ATs")
            CTS = wpool.tile([P, NT * N], F16, tag="CTS")
            BT = wpool.tile([P, NT * 16], F32, tag="BT")
            # round-r strip s lands at partitions 32s..32s+8, free r*N..(r+1)*N
            OUTF = wpool.tile([P, NR * N], F32, tag="OUTF")
            EXF = wpool.tile([P, NR * N], F32, tag="EXF")
            OUT3 = wpool.tile([P, NR * N], F16, tag="OUT3")

            def compute_body():
                # engine busy estimate: ACT pre-loaded with table-load +
                # slab-encoder + softplus work it must do regardless.
                # ACT pre-load: table load 2.6us + slab 1.5 + encoder 2.3 +
                # exp/ln drain 7.2us
                bal = _Balancer({"v": 0.0, "a": 14700.0}
                                | ({"p": 0.0} if USE_GPSIMD else {}))
                R_COST = {"v": 225.0, "a": 690.0} \
                    | ({"p": 15130.0} if USE_GPSIMD else {})
                CP_COST = {"v": 1192.0, "a": 997.0}
                # GPSIMD cannot access PSUM: drains on DVE/ACT only
                DR_COST = {"v": 1192.0, "a": 900.0}

                def r_produce(dst, src, bias_col):
                    e = bal.pick(R_COST)
                    if e == "v":
                        nc.vector.tensor_scalar(dst, src, bias_col, 0.0,
                                                ALU.add, ALU.max)
                    elif e == "a":
                        nc.scalar.activation(dst, src, AF.Relu, bias=bias_col)
                    else:
                        nc.gpsimd.tensor_scalar(dst, src, bias_col, 0.0,
                                                ALU.add, ALU.max)

                # ---------- slab encoder first (feeds the BT bounce) ----------
                with tc.tile_pool(name="eps", bufs=4, space="PSUM") as pps:
                    for mh in range(2):
                        ps = pps.tile([P, SLAB], F32, tag="sps")
                        for kd in range(4):
                            nc.tensor.matmul(
                                ps[:],
                                W1s[:, kd * HID + mh * P: kd * HID + (mh + 1) * P],
                                ETs[:, kd * SLAB:(kd + 1) * SLAB],
                                start=(kd == 0), stop=(kd == 3))
                        nc.scalar.activation(
                            H1Ts[:, mh * SLAB:(mh + 1) * SLAB],
                            ps[:], AF.Relu, bias=b1c[:, mh:mh + 1])
                    for mh in range(2):
                        ps = pps.tile([P, SLAB], F32, tag="sps")
                        for kh in range(2):
                            nc.tensor.matmul(
                                ps[:],
                                W2s[:, kh * HID + mh * P: kh * HID + (mh + 1) * P],
                                H1Ts[:, kh * SLAB:(kh + 1) * SLAB],
                                start=(kh == 0), stop=(kh == 1))
                        nc.scalar.activation(
                            HnTs[:, mh * SLAB:(mh + 1) * SLAB],
                            ps[:], AF.Relu, bias=b2c[:, mh:mh + 1])
                    for mh in range(2):
                        ps = pps.tile([P, SLAB], F32, tag="sps")
                        for kh in range(2):
                            nc.tensor.matmul(
                                ps[:],
                                Was[:, kh * HID + mh * P: kh * HID + (mh + 1) * P],
                                HnTs[:, kh * SLAB:(kh + 1) * SLAB],
                                start=(kh == 0), stop=(kh == 1))
                        nc.scalar.activation(
                            ATs[:, mh * SLAB:(mh + 1) * SLAB],
                            ps[:], AF.Identity, bias=bp1c[:, mh:mh + 1])

                    # BT via DRAM bounce (partition regroup)
                    for mh in range(2):
                        nc.sync.dma_start(ATd[mh * P:(mh + 1) * P, :],
                                          ATs[:, mh * SLAB:(mh + 1) * SLAB])
                    atd_v = ATd[:].rearrange("(t u) (gg c) -> gg u t c",
                                             u=HSUB, gg=G)
                    for g in range(G):
                        dst = BT[g * HSUB:(g + 1) * HSUB, :] \
                            .rearrange("u (t c) -> u t c", c=16)
                        nc.sync.dma_start(dst, atd_v[g])

                    # ---------- full encoder ----------
                    for mh in range(2):
                        for jt in range(2):
                            ps = pps.tile([P, 512], F32, tag="eps")
                            for kd in range(4):
                                nc.tensor.matmul(
                                    ps[:],
                                    W1s[:, kd * HID + mh * P: kd * HID + (mh + 1) * P],
                                    ET[:, kd * N + jt * 512: kd * N + (jt + 1) * 512],
                                    start=(kd == 0), stop=(kd == 3))
                            dstv = H1T[:, mh * N + jt * 512: mh * N + (jt + 1) * 512]
                            if jt == 0:
                                nc.scalar.activation(dstv, ps[:], AF.Relu,
                                                     bias=b1c[:, mh:mh + 1])
                            else:
                                nc.vector.tensor_scalar(dstv, ps[:], b1c[:, mh:mh + 1],
                                                        0.0, ALU.add, ALU.max)
                    for mh in range(2):
                        for jt in range(2):
                            ps = pps.tile([P, 512], F32, tag="eps")
                            for kh in range(2):
                                nc.tensor.matmul(
                                    ps[:],
                                    W2s[:, kh * HID + mh * P: kh * HID + (mh + 1) * P],
                                    H1T[:, kh * N + jt * 512: kh * N + (jt + 1) * 512],
                                    start=(kh == 0), stop=(kh == 1))
                            dstv = HnT[:, mh * N + jt * 512: mh * N + (jt + 1) * 512]
                            if jt == 0:
                                nc.scalar.activation(dstv, ps[:], AF.Relu,
                                                     bias=b2c[:, mh:mh + 1])
                            else:
                                nc.vector.tensor_scalar(dstv, ps[:], b2c[:, mh:mh + 1],
                                                        0.0, ALU.add, ALU.max)

                # ---------- CTS production interleaved with round-0 main.
                # CTS scratch borrows PS4's round-2/3 ranges (rounds 2-3
                # start long after all CTS chunks have been copied out;
                # Tile's WAR tracking serializes the reuse).
                with tc.tile_pool(name="mps", bufs=1, space="PSUM") as mpool:
                    PS4 = mpool.tile([P, NR * N], F32, tag="PS4")

                    def cts_chunk(t):
                        base = (2 + t % 2) * N
                        ps = PS4[:, base:base + N]
                        for jt in range(2):
                            for kh in range(2):
                                nc.tensor.matmul(
                                    ps[:, jt * 512:(jt + 1) * 512],
                                    Wbrep[:, (t * 2 + kh) * P:(t * 2 + kh + 1) * P],
                                    HnT[:, kh * N + jt * 512: kh * N + (jt + 1) * 512],
                                    start=(kh == 0), stop=(kh == 1))
                        dst = CTS[:, t * N:(t + 1) * N]
                        if bal.pick(CP_COST) == "a":
                            nc.scalar.copy(dst, ps)
                        else:
                            nc.vector.tensor_copy(dst, ps)

                    def main_quad(r, t):
                        PS = PS4
                        Rt = []
                        for s in range(NS):
                            R = rpool.tile([P, N], F16, tag="R")
                            bias_col = BT[:, t * 16 + r * 4 + s:
                                          t * 16 + r * 4 + s + 1]
                            r_produce(R[:], CTS[:, t * N:(t + 1) * N], bias_col)
                            Rt.append(R)
                        for jt in range(N // N_MM):
                            for s in range(NS):
                                nc.tensor.matmul(
                                    PS[32 * s:32 * s + 32,
                                       r * N + jt * N_MM: r * N + (jt + 1) * N_MM],
                                    Wst[:, t * 32:(t + 1) * 32],
                                    Rt[s][:, jt * N_MM:(jt + 1) * N_MM],
                                    start=(t == 0), stop=(t == NT - 1),
                                    tile_position=(0, 32 * s))

                    LEAD = 2
                    for t in range(NT):
                        cts_chunk(t)
                        if t >= LEAD:
                            main_quad(0, t - LEAD)
                    for t in range(NT - LEAD, NT):
                        main_quad(0, t)

                    for r in range(NR):
                        if r > 0:
                            for t in range(NT):
                                main_quad(r, t)
                        PS = PS4[:, r * N:(r + 1) * N]
                        # drain round r: one lane-aligned PSUM->SBUF copy
                        # spanning partitions 0..104 covers all 4 strips
                        # (interleaved garbage lanes are free — engine cost
                        # is free-dim cycles, lane count irrelevant);
                        # compaction happens in the final DRAM DMA instead.
                        PW = P
                        src = PS
                        dst = OUTF[0:PW, r * N:(r + 1) * N]
                        e = bal.pick(DR_COST)
                        if e == "v":
                            nc.vector.tensor_copy(dst, src)
                        elif e == "a":
                            nc.scalar.copy(dst, src)
                        else:
                            nc.gpsimd.tensor_copy(dst, src)
                        # softplus = ln(1 + exp(logits + bp2)); per-round so
                        # it overlaps the next round's compute. Garbage lanes
                        # process junk harmlessly.
                        nc.scalar.activation(EXF[0:PW, r * N:(r + 1) * N],
                                             OUTF[0:PW, r * N:(r + 1) * N],
                                             AF.Exp, bias=bp2c[:, 0:1])
                        nc.scalar.activation(OUT3[0:PW, r * N:(r + 1) * N],
                                             EXF[0:PW, r * N:(r + 1) * N],
                                             AF.Ln, bias=1.0)
                        # Y rows r*32+s*8+g <- OUT3[32s+g, r*N+j]
                        # (SBUF AP dim 0 must be the partition dim: one DMA
                        # per strip)
                        for s in range(NS):
                            nc.sync.dma_start(
                                dY.ap()[r * 32 + s * G: r * 32 + (s + 1) * G, :],
                                OUT3[32 * s:32 * s + G, r * N:(r + 1) * N])

            if repeat == 1:
                compute_body()
            else:
                with tc.For_i(0, repeat, 1):
                    compute_body()

    nc.compile()
    return nc


def _prep_inputs(E, W1, b1, W2, b2, Wp1, bp1, Wp2, bp2):
    f32 = np.float32
    E = np.asarray(E, f32)
    W1 = np.asarray(W1, f32)
    b1 = np.asarray(b1, f32)
    W2 = np.asarray(W2, f32)
    b2 = np.asarray(b2, f32)
    Wp1 = np.asarray(Wp1, f32)
    bp1 = np.asarray(bp1, f32)
    Wp2 = np.asarray(Wp2, f32)
    bp2 = np.asarray(bp2, f32)

    ET = np.ascontiguousarray(E.T)                      # (512, 1024)
    W1s = np.ascontiguousarray(
        W1.reshape(4, P, HID).transpose(1, 0, 2).reshape(P, 4 * HID))
    W2s = np.ascontiguousarray(
        W2.reshape(2, P, HID).transpose(1, 0, 2).reshape(P, 2 * HID))
    Wa, Wb = Wp1[:HID], Wp1[HID:]
    Was = np.ascontiguousarray(
        Wa.reshape(2, P, HID).transpose(1, 0, 2).reshape(P, 2 * HID))
    Wbrep = np.zeros((NT, 2, P, P), np.float16)
    for t in range(NT):
        for kh in range(2):
            Wbrep[t, kh] = np.tile(Wb[kh * P:(kh + 1) * P, t * HSUB:(t + 1) * HSUB],
                                   (1, G))
    b1c = np.ascontiguousarray(b1.reshape(2, P).T)
    b2c = np.ascontiguousarray(b2.reshape(2, P).T)
    bp1c = np.ascontiguousarray(bp1.reshape(2, P).T)

    Wst = np.zeros((P, NT * 32), np.float16)
    w = Wp2[:, 0]
    for t in range(NT):
        for g in range(G):
            for u in range(HSUB):
                Wst[g * HSUB + u, t * 32 + g] = w[t * HSUB + u]

    bp2c = np.full((P, 1), bp2[0], np.float32)
    common = {
        "ET": ET, "W1s": W1s, "W2s": W2s, "Was": Was, "Wbrep": Wbrep,
        "b1c": b1c, "b2c": b2c, "bp1c": bp1c, "bp2c": bp2c, "Wst": Wst,
    }
    in_maps = []
    for k in range(NCORE):
        m = dict(common)
        m["ETs"] = np.ascontiguousarray(E[k * SLAB:(k + 1) * SLAB, :].T)
        in_maps.append(m)
    return in_maps, float(bp2[0])


def kernel(E, W1, b1, W2, b2, Wp1, bp1, Wp2, bp2):
    from concourse.bass_utils import run_bass_kernel_spmd

    if "nc" not in _CACHE:
        _CACHE["nc"] = _build_program()
    nc = _CACHE["nc"]

    in_maps, _ = _prep_inputs(E, W1, b1, W2, b2, Wp1, bp1, Wp2, bp2)
    res = run_bass_kernel_spmd(nc, in_maps, list(range(NCORE)))
    # device writes row p = c*8+g for slab-local i = g*16+c; un-permute
    slabs = [np.asarray(res.results[k]["Y"], np.float32)
             .reshape(16, 8, N).transpose(1, 0, 2).reshape(SLAB, N)
             for k in range(NCORE)]
    out = np.concatenate(slabs, axis=0)
    np.fill_diagonal(out, 0.0)
    return np.ascontiguousarray(out.astype(np.float32))


# revision 8
# speedup vs baseline: 1.0861x; 1.0861x over previous
"""Trainium2 Bass kernel for nn_CausalFFNN (pairwise relu-MLP scores), v2.

Computes: Hn = relu(relu(E@W1+b1)@W2+b2)
          logits[i,j] = relu(Hn[i]@Wa + Hn[j]@Wb + bp1) @ Wp2 + bp2
          out = softplus(logits), diag = 0
Sharding: i-rows split across 8 cores (128 rows each); weights + full E
replicated. Each core computes a (128, 1024) output slab.

v2 vs baseline: col-group-concurrent pairwise matmuls (4 streams), R-tile
production load-balanced across DVE/ACT/GPSIMD, single-pass Softplus drain
with DMA-compacted PSUM strips, f16 output.
"""
import sys
import os
import tempfile
import numpy as np

os.environ["NEURON_COMPILE_CACHE_URL"] = tempfile.mkdtemp(prefix="neuron-cache-")

for _p in ("/opt/trn_rl_repo", "/root/.axon_site/_ro/trn_rl_repo"):
    if os.path.isdir(_p) and _p not in sys.path:
        sys.path.insert(0, _p)

N, D, HID = 1024, 512, 256
NCORE = 8
SLAB = N // NCORE          # 128 i-rows per core
P = 128
G = 8                      # i's per PSUM col strip
HSUB = HID // 16           # 16 h-components per chunk per i
NT = HID // HSUB           # 16 chunks
NR = 4                     # rounds
NS = 4                     # col strips
N_MM = 512                 # moving free-dim per pairwise matmul (PSUM bank cap)
USE_GPSIMD = False

_CACHE = {}


class _Balancer:
    """Greedy per-engine busy-time balancer (build-time scheduling)."""

    def __init__(self, init):
        self.busy = dict(init)

    def pick(self, costs):
        e = min(costs, key=lambda e: self.busy[e] + costs[e])
        self.busy[e] += costs[e]
        return e


def _build_program(repeat=1):
    import concourse.bacc as bacc
    import concourse.mybir as mybir
    from concourse.tile import TileContext

    F32 = mybir.dt.float32
    F32R = mybir.dt.float32r
    F16 = mybir.dt.float16
    AF = mybir.ActivationFunctionType
    ALU = mybir.AluOpType

    nc = bacc.Bacc("TRN2", target_bir_lowering=False, debug=False)

    dET = nc.dram_tensor("ET", [D, N], F32R, kind="ExternalInput")
    dETs = nc.dram_tensor("ETs", [D, SLAB], F32R, kind="ExternalInput")
    dW1s = nc.dram_tensor("W1s", [P, 4 * HID], F32R, kind="ExternalInput")
    dW2s = nc.dram_tensor("W2s", [P, 2 * HID], F32R, kind="ExternalInput")
    dWas = nc.dram_tensor("Was", [P, 2 * HID], F32R, kind="ExternalInput")
    dWbrep = nc.dram_tensor("Wbrep", [NT, 2, P, P], F16, kind="ExternalInput")
    db1 = nc.dram_tensor("b1c", [P, 2], F32, kind="ExternalInput")
    db2 = nc.dram_tensor("b2c", [P, 2], F32, kind="ExternalInput")
    dbp1 = nc.dram_tensor("bp1c", [P, 2], F32, kind="ExternalInput")
    dbp2 = nc.dram_tensor("bp2c", [P, 1], F32, kind="ExternalInput")
    dWst = nc.dram_tensor("Wst", [P, NT * G], F16, kind="ExternalInput")
    dY = nc.dram_tensor("Y", [SLAB, N], F16, kind="ExternalOutput")

    with TileContext(nc) as tc:
        with tc.tile_pool(name="const", bufs=1) as cpool, \
             tc.tile_pool(name="work", bufs=1) as wpool, \
             tc.tile_pool(name="rpool", bufs=16) as rpool, \
             tc.tile_pool(name="dpool", bufs=1, space="DRAM") as dpool:

            # ---------- load constants (small/urgent first) ----------
            W1s = cpool.tile([P, 4 * HID], F32R, tag="W1s")
            nc.sync.dma_start(W1s[:], dW1s.ap())
            W2s = cpool.tile([P, 2 * HID], F32R, tag="W2s")
            nc.sync.dma_start(W2s[:], dW2s.ap())
            Was = cpool.tile([P, 2 * HID], F32R, tag="Was")
            nc.sync.dma_start(Was[:], dWas.ap())
            b1c = cpool.tile([P, 2], F32, tag="b1c")
            nc.sync.dma_start(b1c[:], db1.ap())
            b2c = cpool.tile([P, 2], F32, tag="b2c")
            nc.sync.dma_start(b2c[:], db2.ap())
            bp1c = cpool.tile([P, 2], F32, tag="bp1c")
            nc.sync.dma_start(bp1c[:], dbp1.ap())
            bp2c = cpool.tile([P, 1], F32, tag="bp2c")
            nc.sync.dma_start(bp2c[:], dbp2.ap())
            Wst = cpool.tile([P, NT * G], F16, tag="Wst")
            nc.sync.dma_start(Wst[:], dWst.ap())
            ETs = cpool.tile([P, 4 * SLAB], F32R, tag="ETs")
            for kd in range(4):
                nc.sync.dma_start(ETs[:, kd * SLAB:(kd + 1) * SLAB],
                                  dETs.ap()[kd * P:(kd + 1) * P, :])
            ET = cpool.tile([P, 4 * N], F32R, tag="ET")
            for kd in range(4):
                nc.sync.dma_start(ET[:, kd * N:(kd + 1) * N],
                                  dET.ap()[kd * P:(kd + 1) * P, :])
            Wbrep = cpool.tile([P, NT * 2 * P], F16, tag="Wbrep")
            nc.sync.dma_start(
                Wbrep[:].rearrange("p (t kh m) -> p t kh m", kh=2, m=P),
                dWbrep.ap().rearrange("t kh p m -> p t kh m"))

            ATd = dpool.tile([HID, SLAB], F32, tag="ATd")
            H1T = wpool.tile([P, 2 * N], F32R, tag="H1T")
            HnT = wpool.tile([P, 2 * N], F16, tag="HnT")
            H1Ts = wpool.tile([P, 2 * SLAB], F32R, tag="H1Ts")
            HnTs = wpool.tile([P, 2 * SLAB], F32R, tag="HnTs")
            ATs = wpool.tile([P, 2 * SLAB], F32, tag="ATs")
            CTS = wpool.tile([P, NT * N], F16, tag="CTS")
            BT = wpool.tile([P, NT * 16], F32, tag="BT")
            # round-r strip s lands at partitions 32s..32s+8, free r*N..(r+1)*N
            OUTF = wpool.tile([P, NR * N], F32, tag="OUTF")
            EXF = wpool.tile([P, NR * N], F32, tag="EXF")
            OUT3 = wpool.tile([P, NR * N], F16, tag="OUT3")

            def compute_body():
                # engine busy estimate: ACT pre-loaded with table-load +
                # slab-encoder + softplus work it must do regardless.
                # ACT pre-load: table load 2.6us + slab 1.5 + encoder 2.3 +
                # exp/ln drain 7.2us
                bal = _Balancer({"v": 0.0, "a": 14700.0}
                                | ({"p": 0.0} if USE_GPSIMD else {}))
                R_COST = {"v": 225.0, "a": 690.0} \
                    | ({"p": 15130.0} if USE_GPSIMD else {})
                CP_COST = {"v": 1192.0, "a": 997.0}
                # GPSIMD cannot access PSUM: drains on DVE/ACT only
                DR_COST = {"v": 1192.0, "a": 900.0}

                def r_produce(dst, src, bias_col):
                    e = bal.pick(R_COST)
                    if e == "v":
                        nc.vector.tensor_scalar(dst, src, bias_col, 0.0,
                                                ALU.add, ALU.max)
                    elif e == "a":
                        nc.scalar.activation(dst, src, AF.Relu, bias=bias_col)
                    else:
                        nc.gpsimd.tensor_scalar(dst, src, bias_col, 0.0,
                                                ALU.add, ALU.max)

                # ---------- slab encoder first (feeds the BT bounce) ----------
                with tc.tile_pool(name="eps", bufs=4, space="PSUM") as pps:
                    for mh in range(2):
                        ps = pps.tile([P, SLAB], F32, tag="sps")
                        for kd in range(4):
                            nc.tensor.matmul(
                                ps[:],
                                W1s[:, kd * HID + mh * P: kd * HID + (mh + 1) * P],
                                ETs[:, kd * SLAB:(kd + 1) * SLAB],
                                start=(kd == 0), stop=(kd == 3))
                        nc.scalar.activation(
                            H1Ts[:, mh * SLAB:(mh + 1) * SLAB],
                            ps[:], AF.Relu, bias=b1c[:, mh:mh + 1])
                    for mh in range(2):
                        ps = pps.tile([P, SLAB], F32, tag="sps")
                        for kh in range(2):
                            nc.tensor.matmul(
                                ps[:],
                                W2s[:, kh * HID + mh * P: kh * HID + (mh + 1) * P],
                                H1Ts[:, kh * SLAB:(kh + 1) * SLAB],
                                start=(kh == 0), stop=(kh == 1))
                        nc.scalar.activation(
                            HnTs[:, mh * SLAB:(mh + 1) * SLAB],
                            ps[:], AF.Relu, bias=b2c[:, mh:mh + 1])
                    for mh in range(2):
                        ps = pps.tile([P, SLAB], F32, tag="sps")
                        for kh in range(2):
                            nc.tensor.matmul(
                                ps[:],
                                Was[:, kh * HID + mh * P: kh * HID + (mh + 1) * P],
                                HnTs[:, kh * SLAB:(kh + 1) * SLAB],
                                start=(kh == 0), stop=(kh == 1))
                        nc.scalar.activation(
                            ATs[:, mh * SLAB:(mh + 1) * SLAB],
                            ps[:], AF.Identity, bias=bp1c[:, mh:mh + 1])

                    # BT via DRAM bounce (partition regroup)
                    for mh in range(2):
                        nc.sync.dma_start(ATd[mh * P:(mh + 1) * P, :],
                                          ATs[:, mh * SLAB:(mh + 1) * SLAB])
                    atd_v = ATd[:].rearrange("(t u) (gg c) -> gg u t c",
                                             u=HSUB, gg=G)
                    for g in range(G):
                        dst = BT[g * HSUB:(g + 1) * HSUB, :] \
                            .rearrange("u (t c) -> u t c", c=16)
                        nc.sync.dma_start(dst, atd_v[g])

                    # ---------- full encoder ----------
                    for mh in range(2):
                        for jt in range(2):
                            ps = pps.tile([P, 512], F32, tag="eps")
                            for kd in range(4):
                                nc.tensor.matmul(
                                    ps[:],
                                    W1s[:, kd * HID + mh * P: kd * HID + (mh + 1) * P],
                                    ET[:, kd * N + jt * 512: kd * N + (jt + 1) * 512],
                                    start=(kd == 0), stop=(kd == 3))
                            dstv = H1T[:, mh * N + jt * 512: mh * N + (jt + 1) * 512]
                            if jt == 0:
                                nc.scalar.activation(dstv, ps[:], AF.Relu,
                                                     bias=b1c[:, mh:mh + 1])
                            else:
                                nc.vector.tensor_scalar(dstv, ps[:], b1c[:, mh:mh + 1],
                                                        0.0, ALU.add, ALU.max)
                    for mh in range(2):
                        for jt in range(2):
                            ps = pps.tile([P, 512], F32, tag="eps")
                            for kh in range(2):
                                nc.tensor.matmul(
                                    ps[:],
                                    W2s[:, kh * HID + mh * P: kh * HID + (mh + 1) * P],
                                    H1T[:, kh * N + jt * 512: kh * N + (jt + 1) * 512],
                                    start=(kh == 0), stop=(kh == 1))
                            dstv = HnT[:, mh * N + jt * 512: mh * N + (jt + 1) * 512]
                            if jt == 0:
                                nc.scalar.activation(dstv, ps[:], AF.Relu,
                                                     bias=b2c[:, mh:mh + 1])
                            else:
                                nc.vector.tensor_scalar(dstv, ps[:], b2c[:, mh:mh + 1],
                                                        0.0, ALU.add, ALU.max)

                # ---------- CTS production, then pairwise main loop ----------
                with tc.tile_pool(name="cps", bufs=2, space="PSUM") as cpps:

                    def cts_chunk(t):
                        ps = cpps.tile([P, 1024], F32, tag="cps")
                        for jt in range(2):
                            for kh in range(2):
                                nc.tensor.matmul(
                                    ps[:, jt * 512:(jt + 1) * 512],
                                    Wbrep[:, (t * 2 + kh) * P:(t * 2 + kh + 1) * P],
                                    HnT[:, kh * N + jt * 512: kh * N + (jt + 1) * 512],
                                    start=(kh == 0), stop=(kh == 1))
                        dst = CTS[:, t * N:(t + 1) * N]
                        if bal.pick(CP_COST) == "a":
                            nc.scalar.copy(dst, ps[:])
                        else:
                            nc.vector.tensor_copy(dst, ps[:])

                    PS4 = None

                    def main_quad(r, t):
                        PS = PS4
                        Rt = []
                        for s in range(NS):
                            R = rpool.tile([P, N], F16, tag="R")
                            bias_col = BT[:, t * 16 + r * 4 + s:
                                          t * 16 + r * 4 + s + 1]
                            r_produce(R[:], CTS[:, t * N:(t + 1) * N], bias_col)
                            Rt.append(R)
                        for jt in range(N // N_MM):
                            for s in range(NS):
                                nc.tensor.matmul(
                                    PS[32 * s:32 * s + G,
                                       r * N + jt * N_MM: r * N + (jt + 1) * N_MM],
                                    Wst[:, t * G:(t + 1) * G],
                                    Rt[s][:, jt * N_MM:(jt + 1) * N_MM],
                                    start=(t == 0), stop=(t == NT - 1),
                                    tile_position=(0, 32 * s))

                    for t in range(NT):
                        cts_chunk(t)

                with tc.tile_pool(name="mps", bufs=1, space="PSUM") as mpool:
                    PS4 = mpool.tile([P, NR * N], F32, tag="PS4")
                    for r in range(NR):
                        for t in range(NT):
                            main_quad(r, t)
                        PS = PS4[:, r * N:(r + 1) * N]
                        # drain round r: one lane-aligned PSUM->SBUF copy
                        # spanning partitions 0..104 covers all 4 strips
                        # (interleaved garbage lanes are free — engine cost
                        # is free-dim cycles, lane count irrelevant);
                        # compaction happens in the final DRAM DMA instead.
                        PW = P
                        src = PS
                        dst = OUTF[0:PW, r * N:(r + 1) * N]
                        e = bal.pick(DR_COST)
                        if e == "v":
                            nc.vector.tensor_copy(dst, src)
                        elif e == "a":
                            nc.scalar.copy(dst, src)
                        else:
                            nc.gpsimd.tensor_copy(dst, src)
                        # softplus = ln(1 + exp(logits + bp2)); per-round so
                        # it overlaps the next round's compute. Garbage lanes
                        # process junk harmlessly.
                        nc.scalar.activation(EXF[0:PW, r * N:(r + 1) * N],
                                             OUTF[0:PW, r * N:(r + 1) * N],
                                             AF.Exp, bias=bp2c[:, 0:1])
                        nc.scalar.activation(OUT3[0:PW, r * N:(r + 1) * N],
                                             EXF[0:PW, r * N:(r + 1) * N],
                                             AF.Ln, bias=1.0)
                        # Y rows r*32+s*8+g <- OUT3[32s+g, r*N+j]
                        # (SBUF AP dim 0 must be the partition dim: one DMA
                        # per strip)
                        for s in range(NS):
                            nc.sync.dma_start(
                                dY.ap()[r * 32 + s * G: r * 32 + (s + 1) * G, :],
                                OUT3[32 * s:32 * s + G, r * N:(r + 1) * N])

            if repeat == 1:
                compute_body()
            else:
                with tc.For_i(0, repeat, 1):
                    compute_body()

    nc.compile()
    return nc


def _prep_inputs(E, W1, b1, W2, b2, Wp1, bp1, Wp2, bp2):
    f32 = np.float32
    E = np.asarray(E, f32)
    W1 = np.asarray(W1, f32)
    b1 = np.asarray(b1, f32)
    W2 = np.asarray(W2, f32)
    b2 = np.asarray(b2, f32)
    Wp1 = np.asarray(Wp1, f32)
    bp1 = np.asarray(bp1, f32)
    Wp2 = np.asarray(Wp2, f32)
    bp2 = np.asarray(bp2, f32)

    ET = np.ascontiguousarray(E.T)                      # (512, 1024)
    W1s = np.ascontiguousarray(
        W1.reshape(4, P, HID).transpose(1, 0, 2).reshape(P, 4 * HID))
    W2s = np.ascontiguousarray(
        W2.reshape(2, P, HID).transpose(1, 0, 2).reshape(P, 2 * HID))
    Wa, Wb = Wp1[:HID], Wp1[HID:]
    Was = np.ascontiguousarray(
        Wa.reshape(2, P, HID).transpose(1, 0, 2).reshape(P, 2 * HID))
    Wbrep = np.zeros((NT, 2, P, P), np.float16)
    for t in range(NT):
        for kh in range(2):
            Wbrep[t, kh] = np.tile(Wb[kh * P:(kh + 1) * P, t * HSUB:(t + 1) * HSUB],
                                   (1, G))
    b1c = np.ascontiguousarray(b1.reshape(2, P).T)
    b2c = np.ascontiguousarray(b2.reshape(2, P).T)
    bp1c = np.ascontiguousarray(bp1.reshape(2, P).T)

    Wst = np.zeros((P, NT * G), np.float16)
    w = Wp2[:, 0]
    for t in range(NT):
        for g in range(G):
            for u in range(HSUB):
                Wst[g * HSUB + u, t * G + g] = w[t * HSUB + u]

    bp2c = np.full((P, 1), bp2[0], np.float32)
    common = {
        "ET": ET, "W1s": W1s, "W2s": W2s, "Was": Was, "Wbrep": Wbrep,
        "b1c": b1c, "b2c": b2c, "bp1c": bp1c, "bp2c": bp2c, "Wst": Wst,
    }
    in_maps = []
    for k in range(NCORE):
        m = dict(common)
        m["ETs"] = np.ascontiguousarray(E[k * SLAB:(k + 1) * SLAB, :].T)
        in_maps.append(m)
    return in_maps, float(bp2[0])


def kernel(E, W1, b1, W2, b2, Wp1, bp1, Wp2, bp2):
    from concourse.bass_utils import run_bass_kernel_spmd

    if "nc" not in _CACHE:
        _CACHE["nc"] = _build_program()
    nc = _CACHE["nc"]

    in_maps, _ = _prep_inputs(E, W1, b1, W2, b2, Wp1, bp1, Wp2, bp2)
    res = run_bass_kernel_spmd(nc, in_maps, list(range(NCORE)))
    # device writes row p = c*8+g for slab-local i = g*16+c; un-permute
    slabs = [np.asarray(res.results[k]["Y"], np.float32)
             .reshape(16, 8, N).transpose(1, 0, 2).reshape(SLAB, N)
             for k in range(NCORE)]
    out = np.concatenate(slabs, axis=0)
    np.fill_diagonal(out, 0.0)
    return np.ascontiguousarray(out.astype(np.float32))


# revision 9
# speedup vs baseline: 1.1475x; 1.0565x over previous
"""Trainium2 Bass kernel for nn_CausalFFNN (pairwise relu-MLP scores), v2.

Computes: Hn = relu(relu(E@W1+b1)@W2+b2)
          logits[i,j] = relu(Hn[i]@Wa + Hn[j]@Wb + bp1) @ Wp2 + bp2
          out = softplus(logits), diag = 0
Sharding: i-rows split across 8 cores (128 rows each); weights + full E
replicated. Each core computes a (128, 1024) output slab.

v2 vs baseline: col-group-concurrent pairwise matmuls (4 streams), R-tile
production load-balanced across DVE/ACT/GPSIMD, single-pass Softplus drain
with DMA-compacted PSUM strips, f16 output.
"""
import sys
import os
import tempfile
import numpy as np

os.environ["NEURON_COMPILE_CACHE_URL"] = tempfile.mkdtemp(prefix="neuron-cache-")

for _p in ("/opt/trn_rl_repo", "/root/.axon_site/_ro/trn_rl_repo"):
    if os.path.isdir(_p) and _p not in sys.path:
        sys.path.insert(0, _p)

N, D, HID = 1024, 512, 256
NCORE = 8
SLAB = N // NCORE          # 128 i-rows per core
P = 128
G = 8                      # i's per PSUM col strip
HSUB = HID // 16           # 16 h-components per chunk per i
NT = HID // HSUB           # 16 chunks
NR = 4                     # rounds
NS = 4                     # col strips
N_MM = 512                 # moving free-dim per pairwise matmul (PSUM bank cap)
USE_GPSIMD = False

_CACHE = {}


class _Balancer:
    """Greedy per-engine busy-time balancer (build-time scheduling)."""

    def __init__(self, init):
        self.busy = dict(init)

    def pick(self, costs):
        e = min(costs, key=lambda e: self.busy[e] + costs[e])
        self.busy[e] += costs[e]
        return e


def _build_program(repeat=1):
    import concourse.bacc as bacc
    import concourse.mybir as mybir
    from concourse.tile import TileContext

    F32 = mybir.dt.float32
    F32R = mybir.dt.float32r
    F16 = mybir.dt.float16
    AF = mybir.ActivationFunctionType
    ALU = mybir.AluOpType

    nc = bacc.Bacc("TRN2", target_bir_lowering=False, debug=False)

    dET = nc.dram_tensor("ET", [D, N], F32R, kind="ExternalInput")
    dETs = nc.dram_tensor("ETs", [D, SLAB], F32R, kind="ExternalInput")
    dW1s = nc.dram_tensor("W1s", [P, 4 * HID], F32R, kind="ExternalInput")
    dW2s = nc.dram_tensor("W2s", [P, 2 * HID], F32R, kind="ExternalInput")
    dWas = nc.dram_tensor("Was", [P, 2 * HID], F32R, kind="ExternalInput")
    dWbrep = nc.dram_tensor("Wbrep", [NT, 2, P, P], F32R, kind="ExternalInput")
    db1 = nc.dram_tensor("b1c", [P, 2], F32, kind="ExternalInput")
    db2 = nc.dram_tensor("b2c", [P, 2], F32, kind="ExternalInput")
    dbp1 = nc.dram_tensor("bp1c", [P, 2], F32, kind="ExternalInput")
    dbp2 = nc.dram_tensor("bp2c", [P, 1], F32, kind="ExternalInput")
    dWst = nc.dram_tensor("Wst", [P, NT * G], F16, kind="ExternalInput")
    dY = nc.dram_tensor("Y", [SLAB, N], F16, kind="ExternalOutput")

    with TileContext(nc) as tc:
        with tc.tile_pool(name="const", bufs=1) as cpool, \
             tc.tile_pool(name="work", bufs=1) as wpool, \
             tc.tile_pool(name="rpool", bufs=16) as rpool, \
             tc.tile_pool(name="dpool", bufs=1, space="DRAM") as dpool:

            # ---------- load constants (small/urgent first) ----------
            W1s = cpool.tile([P, 4 * HID], F32R, tag="W1s")
            nc.sync.dma_start(W1s[:], dW1s.ap())
            W2s = cpool.tile([P, 2 * HID], F32R, tag="W2s")
            nc.sync.dma_start(W2s[:], dW2s.ap())
            Was = cpool.tile([P, 2 * HID], F32R, tag="Was")
            nc.sync.dma_start(Was[:], dWas.ap())
            b1c = cpool.tile([P, 2], F32, tag="b1c")
            nc.sync.dma_start(b1c[:], db1.ap())
            b2c = cpool.tile([P, 2], F32, tag="b2c")
            nc.sync.dma_start(b2c[:], db2.ap())
            bp1c = cpool.tile([P, 2], F32, tag="bp1c")
            nc.sync.dma_start(bp1c[:], dbp1.ap())
            bp2c = cpool.tile([P, 1], F32, tag="bp2c")
            nc.sync.dma_start(bp2c[:], dbp2.ap())
            Wst = cpool.tile([P, NT * G], F16, tag="Wst")
            nc.sync.dma_start(Wst[:], dWst.ap())
            ETs = cpool.tile([P, 4 * SLAB], F32R, tag="ETs")
            for kd in range(4):
                nc.sync.dma_start(ETs[:, kd * SLAB:(kd + 1) * SLAB],
                                  dETs.ap()[kd * P:(kd + 1) * P, :])
            ET = cpool.tile([P, 4 * N], F32R, tag="ET")
            for kd in range(4):
                nc.sync.dma_start(ET[:, kd * N:(kd + 1) * N],
                                  dET.ap()[kd * P:(kd + 1) * P, :])
            Wbrep = cpool.tile([P, NT * 2 * P], F32R, tag="Wbrep")
            nc.sync.dma_start(
                Wbrep[:].rearrange("p (t kh m) -> p t kh m", kh=2, m=P),
                dWbrep.ap().rearrange("t kh p m -> p t kh m"))

            ATd = dpool.tile([HID, SLAB], F32, tag="ATd")
            H1T = wpool.tile([P, 2 * N], F32R, tag="H1T")
            HnT = wpool.tile([P, 2 * N], F32R, tag="HnT")
            H1Ts = wpool.tile([P, 2 * SLAB], F32R, tag="H1Ts")
            HnTs = wpool.tile([P, 2 * SLAB], F32R, tag="HnTs")
            ATs = wpool.tile([P, 2 * SLAB], F32, tag="ATs")
            CTS = wpool.tile([P, NT * N], F16, tag="CTS")
            BT = wpool.tile([P, NT * 16], F32, tag="BT")
            # round-r strip s lands at partitions 32s..32s+8, free r*N..(r+1)*N
            OUTF = wpool.tile([P, NR * N], F32, tag="OUTF")
            EXF = wpool.tile([P, NR * N], F32, tag="EXF")
            OUT3 = wpool.tile([P, NR * N], F16, tag="OUT3")

            def compute_body():
                # engine busy estimate: ACT pre-loaded with table-load +
                # slab-encoder + softplus work it must do regardless.
                # ACT pre-load: table load 2.6us + slab 1.5 + encoder 2.3 +
                # exp/ln drain 7.2us
                bal = _Balancer({"v": 0.0, "a": 14700.0}
                                | ({"p": 0.0} if USE_GPSIMD else {}))
                R_COST = {"v": 225.0, "a": 690.0} \
                    | ({"p": 15130.0} if USE_GPSIMD else {})
                CP_COST = {"v": 1192.0, "a": 997.0}
                # GPSIMD cannot access PSUM: drains on DVE/ACT only
                DR_COST = {"v": 1192.0, "a": 900.0}

                def r_produce(dst, src, bias_col):
                    e = bal.pick(R_COST)
                    if e == "v":
                        nc.vector.tensor_scalar(dst, src, bias_col, 0.0,
                                                ALU.add, ALU.max)
                    elif e == "a":
                        nc.scalar.activation(dst, src, AF.Relu, bias=bias_col)
                    else:
                        nc.gpsimd.tensor_scalar(dst, src, bias_col, 0.0,
                                                ALU.add, ALU.max)

                # ---------- slab encoder first (feeds the BT bounce) ----------
                with tc.tile_pool(name="eps", bufs=4, space="PSUM") as pps:
                    for mh in range(2):
                        ps = pps.tile([P, SLAB], F32, tag="sps")
                        for kd in range(4):
                            nc.tensor.matmul(
                                ps[:],
                                W1s[:, kd * HID + mh * P: kd * HID + (mh + 1) * P],
                                ETs[:, kd * SLAB:(kd + 1) * SLAB],
                                start=(kd == 0), stop=(kd == 3))
                        nc.scalar.activation(
                            H1Ts[:, mh * SLAB:(mh + 1) * SLAB],
                            ps[:], AF.Relu, bias=b1c[:, mh:mh + 1])
                    for mh in range(2):
                        ps = pps.tile([P, SLAB], F32, tag="sps")
                        for kh in range(2):
                            nc.tensor.matmul(
                                ps[:],
                                W2s[:, kh * HID + mh * P: kh * HID + (mh + 1) * P],
                                H1Ts[:, kh * SLAB:(kh + 1) * SLAB],
                                start=(kh == 0), stop=(kh == 1))
                        nc.scalar.activation(
                            HnTs[:, mh * SLAB:(mh + 1) * SLAB],
                            ps[:], AF.Relu, bias=b2c[:, mh:mh + 1])
                    for mh in range(2):
                        ps = pps.tile([P, SLAB], F32, tag="sps")
                        for kh in range(2):
                            nc.tensor.matmul(
                                ps[:],
                                Was[:, kh * HID + mh * P: kh * HID + (mh + 1) * P],
                                HnTs[:, kh * SLAB:(kh + 1) * SLAB],
                                start=(kh == 0), stop=(kh == 1))
                        nc.scalar.activation(
                            ATs[:, mh * SLAB:(mh + 1) * SLAB],
                            ps[:], AF.Identity, bias=bp1c[:, mh:mh + 1])

                    # BT via DRAM bounce (partition regroup)
                    for mh in range(2):
                        nc.sync.dma_start(ATd[mh * P:(mh + 1) * P, :],
                                          ATs[:, mh * SLAB:(mh + 1) * SLAB])
                    atd_v = ATd[:].rearrange("(t u) (gg c) -> gg u t c",
                                             u=HSUB, gg=G)
                    for g in range(G):
                        dst = BT[g * HSUB:(g + 1) * HSUB, :] \
                            .rearrange("u (t c) -> u t c", c=16)
                        nc.sync.dma_start(dst, atd_v[g])

                    # ---------- full encoder ----------
                    for mh in range(2):
                        for jt in range(2):
                            ps = pps.tile([P, 512], F32, tag="eps")
                            for kd in range(4):
                                nc.tensor.matmul(
                                    ps[:],
                                    W1s[:, kd * HID + mh * P: kd * HID + (mh + 1) * P],
                                    ET[:, kd * N + jt * 512: kd * N + (jt + 1) * 512],
                                    start=(kd == 0), stop=(kd == 3))
                            dstv = H1T[:, mh * N + jt * 512: mh * N + (jt + 1) * 512]
                            if jt == 0:
                                nc.scalar.activation(dstv, ps[:], AF.Relu,
                                                     bias=b1c[:, mh:mh + 1])
                            else:
                                nc.vector.tensor_scalar(dstv, ps[:], b1c[:, mh:mh + 1],
                                                        0.0, ALU.add, ALU.max)
                    for mh in range(2):
                        for jt in range(2):
                            ps = pps.tile([P, 512], F32, tag="eps")
                            for kh in range(2):
                                nc.tensor.matmul(
                                    ps[:],
                                    W2s[:, kh * HID + mh * P: kh * HID + (mh + 1) * P],
                                    H1T[:, kh * N + jt * 512: kh * N + (jt + 1) * 512],
                                    start=(kh == 0), stop=(kh == 1))
                            dstv = HnT[:, mh * N + jt * 512: mh * N + (jt + 1) * 512]
                            if jt == 0:
                                nc.scalar.activation(dstv, ps[:], AF.Relu,
                                                     bias=b2c[:, mh:mh + 1])
                            else:
                                nc.vector.tensor_scalar(dstv, ps[:], b2c[:, mh:mh + 1],
                                                        0.0, ALU.add, ALU.max)

                # ---------- CTS production, then pairwise main loop ----------
                with tc.tile_pool(name="cps", bufs=2, space="PSUM") as cpps:

                    def cts_chunk(t):
                        ps = cpps.tile([P, 1024], F32, tag="cps")
                        for jt in range(2):
                            for kh in range(2):
                                nc.tensor.matmul(
                                    ps[:, jt * 512:(jt + 1) * 512],
                                    Wbrep[:, (t * 2 + kh) * P:(t * 2 + kh + 1) * P],
                                    HnT[:, kh * N + jt * 512: kh * N + (jt + 1) * 512],
                                    start=(kh == 0), stop=(kh == 1))
                        dst = CTS[:, t * N:(t + 1) * N]
                        if bal.pick(CP_COST) == "a":
                            nc.scalar.copy(dst, ps[:])
                        else:
                            nc.vector.tensor_copy(dst, ps[:])

                    PS4 = None

                    def main_quad(r, t):
                        PS = PS4
                        Rt = []
                        for s in range(NS):
                            R = rpool.tile([P, N], F16, tag="R")
                            bias_col = BT[:, t * 16 + r * 4 + s:
                                          t * 16 + r * 4 + s + 1]
                            r_produce(R[:], CTS[:, t * N:(t + 1) * N], bias_col)
                            Rt.append(R)
                        for jt in range(N // N_MM):
                            for s in range(NS):
                                nc.tensor.matmul(
                                    PS[32 * s:32 * s + G,
                                       r * N + jt * N_MM: r * N + (jt + 1) * N_MM],
                                    Wst[:, t * G:(t + 1) * G],
                                    Rt[s][:, jt * N_MM:(jt + 1) * N_MM],
                                    start=(t == 0), stop=(t == NT - 1),
                                    tile_position=(0, 32 * s))

                    for t in range(NT):
                        cts_chunk(t)

                with tc.tile_pool(name="mps", bufs=1, space="PSUM") as mpool:
                    PS4 = mpool.tile([P, NR * N], F32, tag="PS4")
                    for r in range(NR):
                        for t in range(NT):
                            main_quad(r, t)
                        PS = PS4[:, r * N:(r + 1) * N]
                        # drain round r: one lane-aligned PSUM->SBUF copy
                        # spanning partitions 0..104 covers all 4 strips
                        # (interleaved garbage lanes are free — engine cost
                        # is free-dim cycles, lane count irrelevant);
                        # compaction happens in the final DRAM DMA instead.
                        PW = P
                        src = PS
                        dst = OUTF[0:PW, r * N:(r + 1) * N]
                        e = bal.pick(DR_COST)
                        if e == "v":
                            nc.vector.tensor_copy(dst, src)
                        elif e == "a":
                            nc.scalar.copy(dst, src)
                        else:
                            nc.gpsimd.tensor_copy(dst, src)
                        # softplus = ln(1 + exp(logits + bp2)); per-round so
                        # it overlaps the next round's compute. Garbage lanes
                        # process junk harmlessly.
                        nc.scalar.activation(EXF[0:PW, r * N:(r + 1) * N],
                                             OUTF[0:PW, r * N:(r + 1) * N],
                                             AF.Exp, bias=bp2c[:, 0:1])
                        nc.scalar.activation(OUT3[0:PW, r * N:(r + 1) * N],
                                             EXF[0:PW, r * N:(r + 1) * N],
                                             AF.Ln, bias=1.0)
                        # Y rows r*32+s*8+g <- OUT3[32s+g, r*N+j]
                        # (SBUF AP dim 0 must be the partition dim: one DMA
                        # per strip)
                        for s in range(NS):
                            nc.sync.dma_start(
                                dY.ap()[r * 32 + s * G: r * 32 + (s + 1) * G, :],
                                OUT3[32 * s:32 * s + G, r * N:(r + 1) * N])

            if repeat == 1:
                compute_body()
            else:
                with tc.For_i(0, repeat, 1):
                    compute_body()

    nc.compile()
    return nc


def _prep_inputs(E, W1, b1, W2, b2, Wp1, bp1, Wp2, bp2):
    f32 = np.float32
    E = np.asarray(E, f32)
    W1 = np.asarray(W1, f32)
    b1 = np.asarray(b1, f32)
    W2 = np.asarray(W2, f32)
    b2 = np.asarray(b2, f32)
    Wp1 = np.asarray(Wp1, f32)
    bp1 = np.asarray(bp1, f32)
    Wp2 = np.asarray(Wp2, f32)
    bp2 = np.asarray(bp2, f32)

    ET = np.ascontiguousarray(E.T)                      # (512, 1024)
    W1s = np.ascontiguousarray(
        W1.reshape(4, P, HID).transpose(1, 0, 2).reshape(P, 4 * HID))
    W2s = np.ascontiguousarray(
        W2.reshape(2, P, HID).transpose(1, 0, 2).reshape(P, 2 * HID))
    Wa, Wb = Wp1[:HID], Wp1[HID:]
    Was = np.ascontiguousarray(
        Wa.reshape(2, P, HID).transpose(1, 0, 2).reshape(P, 2 * HID))
    Wbrep = np.zeros((NT, 2, P, P), np.float32)
    for t in range(NT):
        for kh in range(2):
            Wbrep[t, kh] = np.tile(Wb[kh * P:(kh + 1) * P, t * HSUB:(t + 1) * HSUB],
                                   (1, G))
    b1c = np.ascontiguousarray(b1.reshape(2, P).T)
    b2c = np.ascontiguousarray(b2.reshape(2, P).T)
    bp1c = np.ascontiguousarray(bp1.reshape(2, P).T)

    Wst = np.zeros((P, NT * G), np.float16)
    w = Wp2[:, 0]
    for t in range(NT):
        for g in range(G):
            for u in range(HSUB):
                Wst[g * HSUB + u, t * G + g] = w[t * HSUB + u]

    bp2c = np.full((P, 1), bp2[0], np.float32)
    common = {
        "ET": ET, "W1s": W1s, "W2s": W2s, "Was": Was, "Wbrep": Wbrep,
        "b1c": b1c, "b2c": b2c, "bp1c": bp1c, "bp2c": bp2c, "Wst": Wst,
    }
    in_maps = []
    for k in range(NCORE):
        m = dict(common)
        m["ETs"] = np.ascontiguousarray(E[k * SLAB:(k + 1) * SLAB, :].T)
        in_maps.append(m)
    return in_maps, float(bp2[0])


def kernel(E, W1, b1, W2, b2, Wp1, bp1, Wp2, bp2):
    from concourse.bass_utils import run_bass_kernel_spmd

    if "nc" not in _CACHE:
        _CACHE["nc"] = _build_program()
    nc = _CACHE["nc"]

    in_maps, _ = _prep_inputs(E, W1, b1, W2, b2, Wp1, bp1, Wp2, bp2)
    res = run_bass_kernel_spmd(nc, in_maps, list(range(NCORE)))
    # device writes row p = c*8+g for slab-local i = g*16+c; un-permute
    slabs = [np.asarray(res.results[k]["Y"], np.float32)
             .reshape(16, 8, N).transpose(1, 0, 2).reshape(SLAB, N)
             for k in range(NCORE)]
    out = np.concatenate(slabs, axis=0)
    np.fill_diagonal(out, 0.0)
    return np.ascontiguousarray(out.astype(np.float32))


# revision 10
# speedup vs baseline: 1.8512x; 1.6132x over previous
"""Trainium2 Bass kernel for nn_CausalFFNN (pairwise relu-MLP scores), v2.

Computes: Hn = relu(relu(E@W1+b1)@W2+b2)
          logits[i,j] = relu(Hn[i]@Wa + Hn[j]@Wb + bp1) @ Wp2 + bp2
          out = softplus(logits), diag = 0
Sharding: i-rows split across 8 cores (128 rows each); weights + full E
replicated. Each core computes a (128, 1024) output slab.

v2 vs baseline: col-group-concurrent pairwise matmuls (4 streams), R-tile
production load-balanced across DVE/ACT/GPSIMD, single-pass Softplus drain
with DMA-compacted PSUM strips, f16 output.
"""
import sys
import os
import tempfile
import numpy as np

os.environ["NEURON_COMPILE_CACHE_URL"] = tempfile.mkdtemp(prefix="neuron-cache-")

for _p in ("/opt/trn_rl_repo", "/root/.axon_site/_ro/trn_rl_repo"):
    if os.path.isdir(_p) and _p not in sys.path:
        sys.path.insert(0, _p)

N, D, HID = 1024, 512, 256
NCORE = 8
SLAB = N // NCORE          # 128 i-rows per core
P = 128
G = 8                      # i's per PSUM col strip
HSUB = HID // 16           # 16 h-components per chunk per i
NT = HID // HSUB           # 16 chunks
NR = 4                     # rounds
NS = 4                     # col strips
N_MM = 512                 # moving free-dim per pairwise matmul (PSUM bank cap)
USE_GPSIMD = False

_CACHE = {}


class _Balancer:
    """Greedy per-engine busy-time balancer (build-time scheduling)."""

    def __init__(self, init):
        self.busy = dict(init)

    def pick(self, costs):
        e = min(costs, key=lambda e: self.busy[e] + costs[e])
        self.busy[e] += costs[e]
        return e


def _build_program(repeat=1):
    import concourse.bacc as bacc
    import concourse.mybir as mybir
    from concourse.tile import TileContext

    F32 = mybir.dt.float32
    F32R = mybir.dt.float32r
    F16 = mybir.dt.float16
    AF = mybir.ActivationFunctionType
    ALU = mybir.AluOpType

    nc = bacc.Bacc("TRN2", target_bir_lowering=False, debug=False)

    dET = nc.dram_tensor("ET", [D, N], F32R, kind="ExternalInput")
    dETs = nc.dram_tensor("ETs", [D, SLAB], F32R, kind="ExternalInput")
    dW1s = nc.dram_tensor("W1s", [P, 4 * HID], F32R, kind="ExternalInput")
    dW2s = nc.dram_tensor("W2s", [P, 2 * HID], F32R, kind="ExternalInput")
    dWas = nc.dram_tensor("Was", [P, 2 * HID], F32R, kind="ExternalInput")
    dWbrep = nc.dram_tensor("Wbrep", [NT, 2, P, P], F32R, kind="ExternalInput")
    db1 = nc.dram_tensor("b1c", [P, 2], F32, kind="ExternalInput")
    db2 = nc.dram_tensor("b2c", [P, 2], F32, kind="ExternalInput")
    dbp1 = nc.dram_tensor("bp1c", [P, 2], F32, kind="ExternalInput")
    dbp2 = nc.dram_tensor("bp2c", [P, 1], F32, kind="ExternalInput")
    dWst = nc.dram_tensor("Wst", [P, NT * 32], F16, kind="ExternalInput")
    dY = nc.dram_tensor("Y", [SLAB, N], F16, kind="ExternalOutput")

    with TileContext(nc) as tc:
        with tc.tile_pool(name="const", bufs=1) as cpool, \
             tc.tile_pool(name="work", bufs=1) as wpool, \
             tc.tile_pool(name="rpool", bufs=16) as rpool, \
             tc.tile_pool(name="dpool", bufs=1, space="DRAM") as dpool:

            # ---------- load constants (small/urgent first) ----------
            W1s = cpool.tile([P, 4 * HID], F32R, tag="W1s")
            nc.sync.dma_start(W1s[:], dW1s.ap())
            W2s = cpool.tile([P, 2 * HID], F32R, tag="W2s")
            nc.sync.dma_start(W2s[:], dW2s.ap())
            Was = cpool.tile([P, 2 * HID], F32R, tag="Was")
            nc.sync.dma_start(Was[:], dWas.ap())
            b1c = cpool.tile([P, 2], F32, tag="b1c")
            nc.sync.dma_start(b1c[:], db1.ap())
            b2c = cpool.tile([P, 2], F32, tag="b2c")
            nc.sync.dma_start(b2c[:], db2.ap())
            bp1c = cpool.tile([P, 2], F32, tag="bp1c")
            nc.sync.dma_start(bp1c[:], dbp1.ap())
            bp2c = cpool.tile([P, 1], F32, tag="bp2c")
            nc.sync.dma_start(bp2c[:], dbp2.ap())
            Wst = cpool.tile([P, NT * 32], F16, tag="Wst")
            nc.sync.dma_start(Wst[:], dWst.ap())
            ETs = cpool.tile([P, 4 * SLAB], F32R, tag="ETs")
            for kd in range(4):
                nc.sync.dma_start(ETs[:, kd * SLAB:(kd + 1) * SLAB],
                                  dETs.ap()[kd * P:(kd + 1) * P, :])
            ET = cpool.tile([P, 4 * N], F32R, tag="ET")
            for kd in range(4):
                nc.sync.dma_start(ET[:, kd * N:(kd + 1) * N],
                                  dET.ap()[kd * P:(kd + 1) * P, :])
            Wbrep = cpool.tile([P, NT * 2 * P], F32R, tag="Wbrep")
            nc.sync.dma_start(
                Wbrep[:].rearrange("p (t kh m) -> p t kh m", kh=2, m=P),
                dWbrep.ap().rearrange("t kh p m -> p t kh m"))

            ATd = dpool.tile([HID, SLAB], F32, tag="ATd")
            H1T = wpool.tile([P, 2 * N], F32R, tag="H1T")
            HnT = wpool.tile([P, 2 * N], F32R, tag="HnT")
            H1Ts = wpool.tile([P, 2 * SLAB], F32R, tag="H1Ts")
            HnTs = wpool.tile([P, 2 * SLAB], F32R, tag="HnTs")
            ATs = wpool.tile([P, 2 * SLAB], F32, tag="ATs")
            CTS = wpool.tile([P, NT * N], F16, tag="CTS")
            BT = wpool.tile([P, NT * 16], F32, tag="BT")
            # round-r strip s lands at partitions 32s..32s+8, free r*N..(r+1)*N
            OUTF = wpool.tile([P, NR * N], F32, tag="OUTF")
            EXF = wpool.tile([P, NR * N], F32, tag="EXF")
            OUT3 = wpool.tile([P, NR * N], F16, tag="OUT3")

            def compute_body():
                # engine busy estimate: ACT pre-loaded with table-load +
                # slab-encoder + softplus work it must do regardless.
                # ACT pre-load: table load 2.6us + slab 1.5 + encoder 2.3 +
                # exp/ln drain 7.2us
                bal = _Balancer({"v": 0.0, "a": 14700.0}
                                | ({"p": 0.0} if USE_GPSIMD else {}))
                R_COST = {"v": 225.0, "a": 690.0} \
                    | ({"p": 15130.0} if USE_GPSIMD else {})
                CP_COST = {"v": 1192.0, "a": 997.0}
                # GPSIMD cannot access PSUM: drains on DVE/ACT only
                DR_COST = {"v": 1192.0, "a": 900.0}

                def r_produce(dst, src, bias_col):
                    e = bal.pick(R_COST)
                    if e == "v":
                        nc.vector.tensor_scalar(dst, src, bias_col, 0.0,
                                                ALU.add, ALU.max)
                    elif e == "a":
                        nc.scalar.activation(dst, src, AF.Relu, bias=bias_col)
                    else:
                        nc.gpsimd.tensor_scalar(dst, src, bias_col, 0.0,
                                                ALU.add, ALU.max)

                # ---------- slab encoder first (feeds the BT bounce) ----------
                with tc.tile_pool(name="eps", bufs=4, space="PSUM") as pps:
                    for mh in range(2):
                        ps = pps.tile([P, SLAB], F32, tag="sps")
                        for kd in range(4):
                            nc.tensor.matmul(
                                ps[:],
                                W1s[:, kd * HID + mh * P: kd * HID + (mh + 1) * P],
                                ETs[:, kd * SLAB:(kd + 1) * SLAB],
                                start=(kd == 0), stop=(kd == 3))
                        nc.scalar.activation(
                            H1Ts[:, mh * SLAB:(mh + 1) * SLAB],
                            ps[:], AF.Relu, bias=b1c[:, mh:mh + 1])
                    for mh in range(2):
                        ps = pps.tile([P, SLAB], F32, tag="sps")
                        for kh in range(2):
                            nc.tensor.matmul(
                                ps[:],
                                W2s[:, kh * HID + mh * P: kh * HID + (mh + 1) * P],
                                H1Ts[:, kh * SLAB:(kh + 1) * SLAB],
                                start=(kh == 0), stop=(kh == 1))
                        nc.scalar.activation(
                            HnTs[:, mh * SLAB:(mh + 1) * SLAB],
                            ps[:], AF.Relu, bias=b2c[:, mh:mh + 1])
                    for mh in range(2):
                        ps = pps.tile([P, SLAB], F32, tag="sps")
                        for kh in range(2):
                            nc.tensor.matmul(
                                ps[:],
                                Was[:, kh * HID + mh * P: kh * HID + (mh + 1) * P],
                                HnTs[:, kh * SLAB:(kh + 1) * SLAB],
                                start=(kh == 0), stop=(kh == 1))
                        nc.scalar.activation(
                            ATs[:, mh * SLAB:(mh + 1) * SLAB],
                            ps[:], AF.Identity, bias=bp1c[:, mh:mh + 1])

                    # BT via DRAM bounce (partition regroup)
                    for mh in range(2):
                        nc.sync.dma_start(ATd[mh * P:(mh + 1) * P, :],
                                          ATs[:, mh * SLAB:(mh + 1) * SLAB])
                    atd_v = ATd[:].rearrange("(t u) (gg c) -> gg u t c",
                                             u=HSUB, gg=G)
                    for g in range(G):
                        dst = BT[g * HSUB:(g + 1) * HSUB, :] \
                            .rearrange("u (t c) -> u t c", c=16)
                        nc.sync.dma_start(dst, atd_v[g])

                    # ---------- full encoder ----------
                    for mh in range(2):
                        for jt in range(2):
                            ps = pps.tile([P, 512], F32, tag="eps")
                            for kd in range(4):
                                nc.tensor.matmul(
                                    ps[:],
                                    W1s[:, kd * HID + mh * P: kd * HID + (mh + 1) * P],
                                    ET[:, kd * N + jt * 512: kd * N + (jt + 1) * 512],
                                    start=(kd == 0), stop=(kd == 3))
                            dstv = H1T[:, mh * N + jt * 512: mh * N + (jt + 1) * 512]
                            if jt == 0:
                                nc.scalar.activation(dstv, ps[:], AF.Relu,
                                                     bias=b1c[:, mh:mh + 1])
                            else:
                                nc.vector.tensor_scalar(dstv, ps[:], b1c[:, mh:mh + 1],
                                                        0.0, ALU.add, ALU.max)
                    for mh in range(2):
                        for jt in range(2):
                            ps = pps.tile([P, 512], F32, tag="eps")
                            for kh in range(2):
                                nc.tensor.matmul(
                                    ps[:],
                                    W2s[:, kh * HID + mh * P: kh * HID + (mh + 1) * P],
                                    H1T[:, kh * N + jt * 512: kh * N + (jt + 1) * 512],
                                    start=(kh == 0), stop=(kh == 1))
                            dstv = HnT[:, mh * N + jt * 512: mh * N + (jt + 1) * 512]
                            if jt == 0:
                                nc.scalar.activation(dstv, ps[:], AF.Relu,
                                                     bias=b2c[:, mh:mh + 1])
                            else:
                                nc.vector.tensor_scalar(dstv, ps[:], b2c[:, mh:mh + 1],
                                                        0.0, ALU.add, ALU.max)

                # ---------- CTS production, then pairwise main loop ----------
                with tc.tile_pool(name="cps", bufs=2, space="PSUM") as cpps:

                    def cts_chunk(t):
                        ps = cpps.tile([P, 1024], F32, tag="cps")
                        for jt in range(2):
                            for kh in range(2):
                                nc.tensor.matmul(
                                    ps[:, jt * 512:(jt + 1) * 512],
                                    Wbrep[:, (t * 2 + kh) * P:(t * 2 + kh + 1) * P],
                                    HnT[:, kh * N + jt * 512: kh * N + (jt + 1) * 512],
                                    start=(kh == 0), stop=(kh == 1))
                        dst = CTS[:, t * N:(t + 1) * N]
                        if bal.pick(CP_COST) == "a":
                            nc.scalar.copy(dst, ps[:])
                        else:
                            nc.vector.tensor_copy(dst, ps[:])

                    PS4 = None

                    def main_quad(r, t):
                        PS = PS4
                        Rt = []
                        for s in range(NS):
                            R = rpool.tile([P, N], F16, tag="R")
                            bias_col = BT[:, t * 16 + r * 4 + s:
                                          t * 16 + r * 4 + s + 1]
                            r_produce(R[:], CTS[:, t * N:(t + 1) * N], bias_col)
                            Rt.append(R)
                        for jt in range(N // N_MM):
                            for s in range(NS):
                                nc.tensor.matmul(
                                    PS[32 * s:32 * s + 32,
                                       r * N + jt * N_MM: r * N + (jt + 1) * N_MM],
                                    Wst[:, t * 32:(t + 1) * 32],
                                    Rt[s][:, jt * N_MM:(jt + 1) * N_MM],
                                    start=(t == 0), stop=(t == NT - 1),
                                    tile_position=(0, 32 * s))

                    for t in range(NT):
                        cts_chunk(t)

                with tc.tile_pool(name="mps", bufs=1, space="PSUM") as mpool:
                    PS4 = mpool.tile([P, NR * N], F32, tag="PS4")
                    for r in range(NR):
                        for t in range(NT):
                            main_quad(r, t)
                        PS = PS4[:, r * N:(r + 1) * N]
                        # drain round r: one lane-aligned PSUM->SBUF copy
                        # spanning partitions 0..104 covers all 4 strips
                        # (interleaved garbage lanes are free — engine cost
                        # is free-dim cycles, lane count irrelevant);
                        # compaction happens in the final DRAM DMA instead.
                        PW = P
                        src = PS
                        dst = OUTF[0:PW, r * N:(r + 1) * N]
                        e = bal.pick(DR_COST)
                        if e == "v":
                            nc.vector.tensor_copy(dst, src)
                        elif e == "a":
                            nc.scalar.copy(dst, src)
                        else:
                            nc.gpsimd.tensor_copy(dst, src)
                        # softplus = ln(1 + exp(logits + bp2)); per-round so
                        # it overlaps the next round's compute. Garbage lanes
                        # process junk harmlessly.
                        nc.scalar.activation(EXF[0:PW, r * N:(r + 1) * N],
                                             OUTF[0:PW, r * N:(r + 1) * N],
                                             AF.Exp, bias=bp2c[:, 0:1])
                        nc.scalar.activation(OUT3[0:PW, r * N:(r + 1) * N],
                                             EXF[0:PW, r * N:(r + 1) * N],
                                             AF.Ln, bias=1.0)
                        # Y rows r*32+s*8+g <- OUT3[32s+g, r*N+j]
                        # (SBUF AP dim 0 must be the partition dim: one DMA
                        # per strip)
                        for s in range(NS):
                            nc.sync.dma_start(
                                dY.ap()[r * 32 + s * G: r * 32 + (s + 1) * G, :],
                                OUT3[32 * s:32 * s + G, r * N:(r + 1) * N])

            if repeat == 1:
                compute_body()
            else:
                with tc.For_i(0, repeat, 1):
                    compute_body()

    nc.compile()
    return nc


def _prep_inputs(E, W1, b1, W2, b2, Wp1, bp1, Wp2, bp2):
    f32 = np.float32
    E = np.asarray(E, f32)
    W1 = np.asarray(W1, f32)
    b1 = np.asarray(b1, f32)
    W2 = np.asarray(W2, f32)
    b2 = np.asarray(b2, f32)
    Wp1 = np.asarray(Wp1, f32)
    bp1 = np.asarray(bp1, f32)
    Wp2 = np.asarray(Wp2, f32)
    bp2 = np.asarray(bp2, f32)

    ET = np.ascontiguousarray(E.T)                      # (512, 1024)
    W1s = np.ascontiguousarray(
        W1.reshape(4, P, HID).transpose(1, 0, 2).reshape(P, 4 * HID))
    W2s = np.ascontiguousarray(
        W2.reshape(2, P, HID).transpose(1, 0, 2).reshape(P, 2 * HID))
    Wa, Wb = Wp1[:HID], Wp1[HID:]
    Was = np.ascontiguousarray(
        Wa.reshape(2, P, HID).transpose(1, 0, 2).reshape(P, 2 * HID))
    Wbrep = np.zeros((NT, 2, P, P), np.float32)
    for t in range(NT):
        for kh in range(2):
            Wbrep[t, kh] = np.tile(Wb[kh * P:(kh + 1) * P, t * HSUB:(t + 1) * HSUB],
                                   (1, G))
    b1c = np.ascontiguousarray(b1.reshape(2, P).T)
    b2c = np.ascontiguousarray(b2.reshape(2, P).T)
    bp1c = np.ascontiguousarray(bp1.reshape(2, P).T)

    Wst = np.zeros((P, NT * 32), np.float16)
    w = Wp2[:, 0]
    for t in range(NT):
        for g in range(G):
            for u in range(HSUB):
                Wst[g * HSUB + u, t * 32 + g] = w[t * HSUB + u]

    bp2c = np.full((P, 1), bp2[0], np.float32)
    common = {
        "ET": ET, "W1s": W1s, "W2s": W2s, "Was": Was, "Wbrep": Wbrep,
        "b1c": b1c, "b2c": b2c, "bp1c": bp1c, "bp2c": bp2c, "Wst": Wst,
    }
    in_maps = []
    for k in range(NCORE):
        m = dict(common)
        m["ETs"] = np.ascontiguousarray(E[k * SLAB:(k + 1) * SLAB, :].T)
        in_maps.append(m)
    return in_maps, float(bp2[0])


def kernel(E, W1, b1, W2, b2, Wp1, bp1, Wp2, bp2):
    from concourse.bass_utils import run_bass_kernel_spmd

    if "nc" not in _CACHE:
        _CACHE["nc"] = _build_program()
    nc = _CACHE["nc"]

    in_maps, _ = _prep_inputs(E, W1, b1, W2, b2, Wp1, bp1, Wp2, bp2)
    res = run_bass_kernel_spmd(nc, in_maps, list(range(NCORE)))
    # device writes row p = c*8+g for slab-local i = g*16+c; un-permute
    slabs = [np.asarray(res.results[k]["Y"], np.float32)
             .reshape(16, 8, N).transpose(1, 0, 2).reshape(SLAB, N)
             for k in range(NCORE)]
    out = np.concatenate(slabs, axis=0)
    np.fill_diagonal(out, 0.0)
    return np.ascontiguousarray(out.astype(np.float32))
